# revision 1
# baseline (speedup 1.0000x reference)
"""Trainium2 Bass kernel v2 for the dense transformer block, data-parallel over
batch across 8 cores. All-SBUF dataflow (no DRAM round trips for q/k/v/h),
fused per-head attention pipelined with the qkv GEMM, F-quartered fused MLP
with long fc2 PSUM chains, PE-matmul broadcasts instead of DRAM-bounce
broadcasts, DVE/Pool work split, and an Ldweights-dedup pass on the emitted
walrus JSON.
"""

import numpy as np

N_EMBD = 2048
N_HEAD = 16
HEAD_DIM = 128
B, T = 8, 1024
F = 4 * N_EMBD
P = 128
C = N_EMBD
KC = C // P            # 16 k-tiles over C
KF = F // P            # 64 f-tiles over F
KFQ = KF // 4          # 16 f-tiles per F-quarter
NCH = T // 512         # 2 chunks of 512 tokens
VCH = C // 512         # 4 output chunks for v
EPS = 1e-5

DEDUP_LDW = True

# ---------------------------------------------------------------------------
# JSON post-passes: (1) drop redundant Ldweights (same stationary AP, no
# waits, contiguous Ld/Mat run); (2) walrus single-wait fix: split surplus
# sync waits onto same-engine NoOps.
_PATCHED = False


def _apply_patches():
    global _PATCHED
    if _PATCHED:
        return
    _PATCHED = True
    import orjson
    import concourse.tile as _tile
    import concourse.bass as _bass
    import concourse.mybir as mybir
    from concourse.vector_clock import ScopedClock

    def _patched_drain_and_barrier(self, tick_clock, wait_clock):
        drain_inst = self.nc.sync.drain()
        wait_clock.add_sem_waits(
            drain_inst.ins, ScopedClock({None: tick_clock.global_clock})
        )
        si = drain_inst.ins.sync_info
        if si is not None and len(si.on_wait) > 1:
            waits = list(si.on_wait)
            drain_inst.ins.sync_info = mybir.SyncInfo(
                on_wait=[waits[0]], on_update=list(si.on_update)
            )
            for w in waits[1:]:
                nop = self.nc.sync.nop()
                nop.ins.sync_info = mybir.SyncInfo(on_wait=[w], on_update=[])
        self.nc.all_engine_barrier()
        assert self.sems is not None
        popped = self.nc._tile_sem_poison_stack.pop()
        assert popped is self._sem_poison
        self.nc.clear_and_free_semaphores(list(self.sems.allocated().values()))
        self.nc.all_engine_barrier()

    _tile.TileContext._drain_and_barrier = _patched_drain_and_barrier

    _orig_to_json_bytes = _bass.Bass.to_json_bytes

    def _dedup_ldw(j):
        """Remove Ldweights whose stationary operand equals the previous
        Ldweights in a contiguous {Ldweights, Matmult} PE run, provided the
        duplicate carries no sync (so no producer rewrote the buffer)."""
        import orjson as _oj
        for fn in j.get("functions", []):
            for bb in fn.get("blocks", []):
                out = []
                last_sig = None
                for ins in bb.get("instructions", []):
                    if ins["engine"] != "PE":
                        out.append(ins)
                        continue
                    op = ins["opcode"]
                    if op == "Ldweights":
                        si = ins.get("sync_info")
                        nsync = ((len(si.get("on_wait") or [])
                                  + len(si.get("on_update") or []))
                                 if si else 0)
                        sig = _oj.dumps([
                            ins.get("ins"), ins.get("perf_mode"),
                            ins.get("is_transpose"), ins.get("tile_position"),
                            ins.get("tile_size"),
                        ])
                        if sig == last_sig and nsync == 0:
                            continue  # drop duplicate
                        last_sig = sig
                        out.append(ins)
                    elif op == "Matmult":
                        if ins.get("is_transpose") or ins.get("ldweights"):
                            last_sig = None
                        out.append(ins)
                    else:
                        last_sig = None
                        out.append(ins)
                bb["instructions"] = out
        return j

    def _split_waits(j):
        ctr = 0
        for fn in j.get("functions", []):
            for bb in fn.get("blocks", []):
                insts = bb.get("instructions", [])
                out = []
                changed = False
                for ins in insts:
                    si = ins.get("sync_info")
                    waits = si.get("on_wait") if si else None
                    if waits and len(waits) > 1:
                        extra = waits[1:]
                        si["on_wait"] = waits[:1]
                        for w in extra:
                            ctr += 1
                            out.append({
                                "debug": ins.get("debug", 0),
                                "engine": ins["engine"],
                                "ins": [],
                                "name": f"waitnop-{ctr}",
                                "opcode": "NoOp",
                                "outs": [],
                                "sync_info": {"on_update": [], "on_wait": [w]},
                            })
                        changed = True
                    out.append(ins)
                if changed:
                    bb["instructions"] = out
        return j

    def _patched_to_json_bytes(self) -> bytes:
        j = orjson.loads(_orig_to_json_bytes(self))
        if DEDUP_LDW:
            j = _dedup_ldw(j)
        j = _split_waits(j)
        return orjson.dumps(j)

    _bass.Bass.to_json_bytes = _patched_to_json_bytes


# ---------------------------------------------------------------------------
def ts(i, sz):
    return slice(i * sz, (i + 1) * sz)


def build_block_bass(reps: int = 1):
    _apply_patches()
    import contextlib
    import concourse.bass as bass
    import concourse.mybir as mybir
    import concourse.tile as tile

    f32 = mybir.dt.float32
    bf16 = mybir.dt.bfloat16
    ACT = mybir.ActivationFunctionType
    MUL = mybir.AluOpType.mult
    ADD = mybir.AluOpType.add
    SCALE = 1.0 / float(np.sqrt(HEAD_DIM))

    nc = bass.Bass()
    xT = nc.declare_dram_parameter("xT", [C, T], f32, isOutput=False)
    wqk = nc.declare_dram_parameter("wqk", [2 * KC, P, KC, P], bf16,
                                    isOutput=False)
    wv = nc.declare_dram_parameter("wv", [KC, P, VCH, 512], bf16,
                                   isOutput=False)
    wo = nc.declare_dram_parameter("wo", [KC, P, KC, P], bf16, isOutput=False)
    w1 = nc.declare_dram_parameter("w1", [KF, P, KC, P], bf16, isOutput=False)
    w2 = nc.declare_dram_parameter("w2", [KC, 4, P, KFQ, P], bf16,
                                   isOutput=False)
    qkb = nc.declare_dram_parameter("qkb", [P, 2 * KC], f32, isOutput=False)
    vbc = nc.declare_dram_parameter("vbc", [P, N_HEAD], f32, isOutput=False)
    outb = nc.declare_dram_parameter("outb", [P, KC], f32, isOutput=False)
    fc1b = nc.declare_dram_parameter("fc1b", [P, KF], f32, isOutput=False)
    fc2b = nc.declare_dram_parameter("fc2b", [P, KC], f32, isOutput=False)
    ln1w = nc.declare_dram_parameter("ln1w", [P, KC], f32, isOutput=False)
    ln1b = nc.declare_dram_parameter("ln1b", [P, KC], f32, isOutput=False)
    ln2w = nc.declare_dram_parameter("ln2w", [P, KC], f32, isOutput=False)
    ln2b = nc.declare_dram_parameter("ln2b", [P, KC], f32, isOutput=False)
    masks = nc.declare_dram_parameter("masks", [P, 4, 512], bf16,
                                      isOutput=False)
    outT = nc.declare_dram_parameter("outT", [C, T], f32, isOutput=True)

    with tile.TileContext(nc) as tc, contextlib.ExitStack() as ctx:
        const = ctx.enter_context(tc.tile_pool(name="const", bufs=1))
        big = ctx.enter_context(tc.tile_pool(name="big", bufs=1))
        wkp = ctx.enter_context(tc.tile_pool(name="wkp", bufs=2))
        w2p = ctx.enter_context(tc.tile_pool(name="w2p", bufs=2))
        xtp = ctx.enter_context(tc.tile_pool(name="xtp", bufs=2))
        lnp = ctx.enter_context(tc.tile_pool(name="lnp", bufs=2))
        qkp = ctx.enter_context(tc.tile_pool(name="qkp", bufs=4))
        esp = ctx.enter_context(tc.tile_pool(name="esp", bufs=1))
        bcs = ctx.enter_context(tc.tile_pool(name="bcs", bufs=2))
        stp = ctx.enter_context(tc.tile_pool(name="stp", bufs=2))
        pmm = ctx.enter_context(tc.tile_pool(name="pmm", bufs=4, space="PSUM"))
        paux = ctx.enter_context(tc.tile_pool(name="paux", bufs=4,
                                              space="PSUM"))

        # ---------------- constants ----------------
        qkb_sb = const.tile([P, 2 * KC], f32)
        nc.sync.dma_start(out=qkb_sb, in_=qkb[:])
        vbc_sb = const.tile([P, N_HEAD], f32)
        nc.sync.dma_start(out=vbc_sb, in_=vbc[:])
        outb_sb = const.tile([P, KC], f32)
        nc.sync.dma_start(out=outb_sb, in_=outb[:])
        fc1b_sb = const.tile([P, KF], f32)
        nc.sync.dma_start(out=fc1b_sb, in_=fc1b[:])
        fc2b_sb = const.tile([P, KC], f32)
        nc.sync.dma_start(out=fc2b_sb, in_=fc2b[:])
        ln1w_sb = const.tile([P, KC], f32)
        nc.sync.dma_start(out=ln1w_sb, in_=ln1w[:])
        ln1b_sb = const.tile([P, KC], f32)
        nc.sync.dma_start(out=ln1b_sb, in_=ln1b[:])
        ln2w_sb = const.tile([P, KC], f32)
        nc.sync.dma_start(out=ln2w_sb, in_=ln2w[:])
        ln2b_sb = const.tile([P, KC], f32)
        nc.sync.dma_start(out=ln2b_sb, in_=ln2b[:])
        masks_sb = const.tile([P, 4, 512], bf16)
        nc.sync.dma_start(out=masks_sb, in_=masks[:])
        ones_colb = const.tile([P, 1], bf16)
        nc.vector.memset(ones_colb, 1.0)
        ones_row = const.tile([1, P], bf16)
        nc.vector.memset(ones_row, 1.0)

        def eng(i):
            return nc.vector if (i % 2 == 0) else nc.gpsimd

        def ln_finish(mu_ps, sq_ps, tagph):
            """Combine LN stat psums into per-partition-broadcast bf16 rows
            nm_b (=-mu) and iv_b (=1/std), [P, T] in SBUF."""
            sqm = stp.tile([1, T], f32, tag="st", name=f"sqm{tagph}")
            var = stp.tile([1, T], f32, tag="st", name=f"var{tagph}")
            negmub = stp.tile([1, T], bf16, tag="stb", name=f"nm{tagph}")
            for j in range(NCH):
                sl = ts(j, 512)
                nc.scalar.activation(negmub[:, sl], mu_ps[j], ACT.Copy,
                                     scale=-1.0 / C)
                nc.scalar.activation(sqm[:, sl], sq_ps[j], ACT.Copy,
                                     scale=1.0 / C)
            nc.vector.tensor_mul(var, negmub, negmub)
            nc.vector.tensor_sub(var, sqm, var)
            nc.vector.tensor_scalar_add(var, var, EPS)
            nc.vector.reciprocal(var, var)
            invb = stp.tile([1, T], bf16, tag="stb", name=f"inv{tagph}")
            nc.scalar.activation(invb, var, ACT.Sqrt)
            nm_b = bcs.tile([P, T], bf16, tag="bc", name=f"nmb{tagph}")
            iv_b = bcs.tile([P, T], bf16, tag="bc", name=f"ivb{tagph}")
            for j in range(NCH):
                sl = ts(j, 512)
                bp = pmm.tile([P, 512], f32, tag="mm", name=f"bnm{tagph}{j}")
                nc.tensor.matmul(bp, ones_row, negmub[:, sl],
                                 start=True, stop=True)
                nc.scalar.activation(nm_b[:, sl], bp, ACT.Copy)
                bp2 = pmm.tile([P, 512], f32, tag="mm", name=f"biv{tagph}{j}")
                nc.tensor.matmul(bp2, ones_row, invb[:, sl],
                                 start=True, stop=True)
                nc.scalar.activation(iv_b[:, sl], bp2, ACT.Copy)
            return nm_b, iv_b

        def body(it):
            xln = big.tile([P, KC, T], bf16, tag="xln", name="xln")
            vsb = big.tile([P, T // P, C], bf16, tag="vsb", name="vsb")
            att = big.tile([P, KC, T], bf16, tag="hp", name="att")
            res1 = big.tile([P, KC, T], bf16, tag="res", name="res1")

            # ---------------- LN1 (x streamed once; apply in-place) --------
            mu_ps = [paux.tile([1, 512], f32, tag="aux", name=f"mups{j}")
                     for j in range(NCH)]
            sq_ps = [paux.tile([1, 512], f32, tag="aux", name=f"sqps{j}")
                     for j in range(NCH)]
            for k in range(KC):
                xt = xtp.tile([P, T], f32, tag="xt", name=f"x{k}")
                nc.sync.dma_start(out=xt, in_=xT[k * P:(k + 1) * P, :])
                nc.scalar.activation(xln[:, k, :], xt, ACT.Copy)
                sq = lnp.tile([P, T], bf16, tag="ln", name=f"lsq{k}")
                nc.scalar.activation(sq, xt, ACT.Square)
                for j in range(NCH):
                    sl = ts(j, 512)
                    nc.tensor.matmul(mu_ps[j], ones_colb, xln[:, k, sl],
                                     start=(k == 0), stop=(k == KC - 1))
                    nc.tensor.matmul(sq_ps[j], ones_colb, sq[:, sl],
                                     start=(k == 0), stop=(k == KC - 1))
            nm_b, iv_b = ln_finish(mu_ps, sq_ps, "a")
            for k in range(KC):
                e = eng(k)
                e.tensor_add(xln[:, k, :], xln[:, k, :], nm_b)
                e.tensor_mul(xln[:, k, :], xln[:, k, :], iv_b)
                e.tensor_scalar(xln[:, k, :], xln[:, k, :],
                                ln1w_sb[:, k:k + 1], ln1b_sb[:, k:k + 1],
                                MUL, ADD)

            # ---------------- v GEMM (x stationary, wv moving) -------------
            for tg in range(4):
                pss = {}
                for k in range(KC):
                    wvt = wkp.tile([P, VCH, 512], bf16, tag="wv",
                                   name=f"wv{tg}_{k}", bufs=2)
                    nc.sync.dma_start(out=wvt, in_=wv[k])
                    for ti in range(2):
                        tt = tg * 2 + ti
                        for ch in range(VCH):
                            key = (ti, ch)
                            if key not in pss:
                                pool, tg2 = (pmm, "mm") if ti == 0 else (paux, "aux")
                                pss[key] = pool.tile(
                                    [P, 512], f32, tag=tg2,
                                    name=f"vps{tg}_{ti}_{ch}")
                            nc.tensor.matmul(pss[key], xln[:, k, ts(tt, P)],
                                             wvt[:, ch], start=(k == 0),
                                             stop=(k == KC - 1))
                for ti in range(2):
                    tt = tg * 2 + ti
                    for ch in range(VCH):
                        nc.scalar.activation(vsb[:, tt, ts(ch, 512)],
                                             pss[(ti, ch)], ACT.Copy)

            # ---------------- fused qk GEMM + attention -------------------
            qks = {}

            def qk_gemm(h):
                for qi in range(2):  # 0: q head h, 1: k head h
                    o = h + 16 * qi
                    wt = wkp.tile([P, KC, P], bf16, tag="wk", name=f"wqk{o}")
                    nc.sync.dma_start(out=wt, in_=wqk[o])
                    pss = [pmm.tile([P, 512], f32, tag="mm", name=f"qk{o}_{j}")
                           for j in range(NCH)]
                    for k in range(KC):
                        for j in range(NCH):
                            nc.tensor.matmul(pss[j], wt[:, k],
                                             xln[:, k, ts(j, 512)],
                                             start=(k == 0), stop=(k == KC - 1))
                    ev = qkp.tile([P, T], bf16, tag="qh", name=f"ev{o}")
                    for j in range(NCH):
                        nc.vector.tensor_scalar_add(ev[:, ts(j, 512)], pss[j],
                                                    qkb_sb[:, o:o + 1])
                    qks[(h, qi)] = ev

            def attention(h):
                qh = qks.pop((h, 0))
                kh = qks.pop((h, 1))
                for j in range(NCH):
                    ntk = 4 * j + 4
                    es = esp.tile([P, 8, 512], bf16, tag="es",
                                  name=f"es{h}_{j}")
                    for tk in range(ntk):
                        sps = paux.tile([P, 512], f32, tag="aux",
                                        name=f"sps{h}_{j}_{tk}")
                        nc.tensor.matmul(sps, kh[:, ts(tk, P)],
                                         qh[:, ts(j, 512)],
                                         start=True, stop=True)
                        nc.scalar.activation(es[:, tk, :], sps, ACT.Exp,
                                             scale=SCALE)
                        off = tk * P - j * 512
                        if off >= 0:
                            e = eng(h + tk)
                            e.tensor_mul(es[:, tk, :], es[:, tk, :],
                                         masks_sb[:, off // P, :])
                    zps = paux.tile([1, 512], f32, tag="aux",
                                    name=f"zps{h}_{j}")
                    for tk in range(ntk):
                        nc.tensor.matmul(zps, ones_colb, es[:, tk, :],
                                         start=(tk == 0), stop=(tk == ntk - 1))
                    zv = stp.tile([1, 512], bf16, tag="stz", name=f"zv{h}_{j}")
                    with nc.allow_low_precision(reason="bf16 1/Z row"):
                        nc.vector.reciprocal(zv, zps)
                    zbp = paux.tile([P, 512], f32, tag="aux",
                                    name=f"zb{h}_{j}")
                    nc.tensor.matmul(zbp, ones_row, zv, start=True, stop=True)
                    zbs = stp.tile([P, 512], bf16, tag="stb2",
                                   name=f"zbs{h}_{j}")
                    nc.scalar.activation(zbs, zbp, ACT.Copy)
                    ops = pmm.tile([P, 512], f32, tag="mm", name=f"ops{h}_{j}")
                    for tk in range(ntk):
                        nc.tensor.matmul(ops, vsb[:, tk, ts(h, P)],
                                         es[:, tk, :],
                                         start=(tk == 0), stop=(tk == ntk - 1))
                    nc.vector.tensor_mul(att[:, h, ts(j, 512)], ops, zbs)
                    nc.gpsimd.tensor_scalar_add(att[:, h, ts(j, 512)],
                                                att[:, h, ts(j, 512)],
                                                vbc_sb[:, h:h + 1])

            qk_gemm(0)
            for h in range(N_HEAD):
                if h + 1 < N_HEAD:
                    qk_gemm(h + 1)
                attention(h)

            # ---------------- out_proj + residual + LN2 stats --------------
            mu2 = [paux.tile([1, 512], f32, tag="aux", name=f"mu2_{j}")
                   for j in range(NCH)]
            sq2 = [paux.tile([1, 512], f32, tag="aux", name=f"sq2_{j}")
                   for j in range(NCH)]
            for o in range(KC):
                wt = wkp.tile([P, KC, P], bf16, tag="wk", name=f"wo{o}")
                nc.sync.dma_start(out=wt, in_=wo[o])
                xt = xtp.tile([P, T], f32, tag="xt", name=f"xr{o}")
                nc.sync.dma_start(out=xt, in_=xT[o * P:(o + 1) * P, :])
                pss = [pmm.tile([P, 512], f32, tag="mm", name=f"op{o}_{j}")
                       for j in range(NCH)]
                for k in range(KC):
                    for j in range(NCH):
                        nc.tensor.matmul(pss[j], wt[:, k],
                                         att[:, k, ts(j, 512)],
                                         start=(k == 0), stop=(k == KC - 1))
                for j in range(NCH):
                    sl = ts(j, 512)
                    nc.vector.scalar_tensor_tensor(
                        res1[:, o, sl], pss[j], outb_sb[:, o:o + 1],
                        xt[:, sl], ADD, ADD)
                sq = lnp.tile([P, T], bf16, tag="ln", name=f"l2s{o}")
                nc.scalar.activation(sq, res1[:, o, :], ACT.Square)
                for j in range(NCH):
                    sl = ts(j, 512)
                    nc.tensor.matmul(mu2[j], ones_colb, res1[:, o, sl],
                                     start=(o == 0), stop=(o == KC - 1))
                    nc.tensor.matmul(sq2[j], ones_colb, sq[:, sl],
                                     start=(o == 0), stop=(o == KC - 1))

            # ---------------- LN2 finish + apply ---------------------------
            xln2 = big.tile([P, KC, T], bf16, tag="xln", name="xln2")
            nm2_b, iv2_b = ln_finish(mu2, sq2, "b")
            for k in range(KC):
                e = eng(k)
                e.tensor_add(xln2[:, k, :], res1[:, k, :], nm2_b)
                e.tensor_mul(xln2[:, k, :], xln2[:, k, :], iv2_b)
                e.tensor_scalar(xln2[:, k, :], xln2[:, k, :],
                                ln2w_sb[:, k:k + 1], ln2b_sb[:, k:k + 1],
                                MUL, ADD)

            # ---------------- MLP: four F-quarters, fused fc1 -> fc2 -------
            for qt in range(4):
                hq = big.tile([P, KFQ, T], bf16, tag="hp", name=f"hq{qt}")
                for fi in range(KFQ):
                    f = qt * KFQ + fi
                    wt = wkp.tile([P, KC, P], bf16, tag="wk", name=f"w1{f}")
                    nc.sync.dma_start(out=wt, in_=w1[f])
                    pss = [pmm.tile([P, 512], f32, tag="mm",
                                    name=f"h1{f}_{j}") for j in range(NCH)]
                    for k in range(KC):
                        for j in range(NCH):
                            nc.tensor.matmul(pss[j], wt[:, k],
                                             xln2[:, k, ts(j, 512)],
                                             start=(k == 0),
                                             stop=(k == KC - 1))
                    for j in range(NCH):
                        nc.scalar.activation(
                            hq[:, fi, ts(j, 512)], pss[j],
                            ACT.Gelu, bias=fc1b_sb[:, f:f + 1])
                for o in range(KC):
                    wt = w2p.tile([P, KFQ, P], bf16, tag="w2",
                                  name=f"w2{qt}_{o}")
                    nc.sync.dma_start(out=wt, in_=w2[o, qt])
                    pss = [paux.tile([P, 512], f32, tag="aux",
                                     name=f"f2{qt}_{o}_{j}")
                           for j in range(NCH)]
                    for kf in range(KFQ):
                        for j in range(NCH):
                            nc.tensor.matmul(
                                pss[j], wt[:, kf],
                                hq[:, kf, ts(j, 512)],
                                start=(kf == 0), stop=(kf == KFQ - 1))
                    for j in range(NCH):
                        sl = ts(j, 512)
                        if qt == 0:
                            nc.vector.scalar_tensor_tensor(
                                res1[:, o, sl], pss[j], fc2b_sb[:, o:o + 1],
                                res1[:, o, sl], ADD, ADD)
                        else:
                            nc.vector.tensor_add(res1[:, o, sl],
                                                 res1[:, o, sl], pss[j])

            # ---------------- output (ACT widens bf16->f32, sync DMA) -------
            for o in range(KC):
                ot = xtp.tile([P, T], f32, tag="xt", name=f"ot{o}")
                nc.scalar.activation(ot, res1[:, o, :], ACT.Copy)
                nc.sync.dma_start(out=outT[o * P:(o + 1) * P, :], in_=ot)

        if reps > 1:
            with tc.For_i(0, reps, 1) as it:
                body(it)
        else:
            body(0)

    return nc


# ---------------------------------------------------------------------------
def _pack_weights(inputs):
    import ml_dtypes
    bf16 = ml_dtypes.bfloat16
    f32 = np.float32

    qkv_w = np.asarray(inputs["qkv_w"], f32)     # [3C, C]
    out_w = np.asarray(inputs["out_w"], f32)     # [C, C]
    fc1_w = np.asarray(inputs["fc1_w"], f32)     # [F, C]
    fc2_w = np.asarray(inputs["fc2_w"], f32)     # [C, F]

    WqkT = qkv_w[:2 * C, :].T                    # [C, 2C]
    wqk = np.ascontiguousarray(
        WqkT.reshape(KC, P, 2 * KC, P).transpose(2, 1, 0, 3)).astype(bf16)
    WvT = qkv_w[2 * C:, :].T                     # [C, C] (c_in, c_out)
    wv = np.ascontiguousarray(WvT.reshape(KC, P, VCH, 512)).astype(bf16)
    WoT = out_w.T                                # [C, C]
    wo = np.ascontiguousarray(
        WoT.reshape(KC, P, KC, P).transpose(2, 1, 0, 3)).astype(bf16)
    W1T = fc1_w.T                                # [C, F]
    w1 = np.ascontiguousarray(
        W1T.reshape(KC, P, KF, P).transpose(2, 1, 0, 3)).astype(bf16)
    W2T = fc2_w.T                                # [F, C]
    # w2[o, qt, p, kf, q] = W2T[(qt*KFQ+kf)*P + p, o*P + q]
    w2 = np.ascontiguousarray(
        W2T.reshape(4, KFQ, P, KC, P).transpose(3, 0, 2, 1, 4)).astype(bf16)

    qkv_b = np.asarray(inputs["qkv_b"], f32)
    qkb = np.ascontiguousarray(qkv_b[:2 * C].reshape(2 * KC, P).T)
    vbc = np.ascontiguousarray(qkv_b[2 * C:].reshape(N_HEAD, P).T)

    def colpack(b, n):
        return np.ascontiguousarray(np.asarray(b, f32).reshape(n, P).T)

    packs = {
        "wqk": wqk, "wv": wv, "wo": wo, "w1": w1, "w2": w2,
        "qkb": qkb, "vbc": vbc,
        "outb": colpack(inputs["out_b"], KC),
        "fc1b": colpack(inputs["fc1_b"], KF),
        "fc2b": colpack(inputs["fc2_b"], KC),
        "ln1w": colpack(inputs["ln1_w"], KC),
        "ln1b": colpack(inputs["ln1_b"], KC),
        "ln2w": colpack(inputs["ln2_w"], KC),
        "ln2b": colpack(inputs["ln2_b"], KC),
    }
    tk = np.arange(P)[:, None, None]
    oi = np.arange(4)[None, :, None] * P
    tq = np.arange(512)[None, None, :]
    packs["masks"] = ((tk + oi) <= tq).astype(bf16)
    return packs


_NC_CACHE = {}


def _get_nc(reps=1):
    if reps not in _NC_CACHE:
        _NC_CACHE[reps] = build_block_bass(reps)
    return _NC_CACHE[reps]


def run_spmd(inputs, reps=1):
    _apply_patches()
    from concourse.bass_utils import run_bass_kernel_spmd
    nc = _get_nc(reps)
    packs = _pack_weights(inputs)
    x = np.asarray(inputs["x"], np.float32)
    in_maps = []
    for b in range(B):
        m = dict(packs)
        m["xT"] = np.ascontiguousarray(x[b].T)
        in_maps.append(m)
    res = run_bass_kernel_spmd(nc, in_maps, list(range(B)))
    out = np.stack([np.ascontiguousarray(res.results[b]["outT"].T)
                    for b in range(B)])
    return out


def kernel(**inputs) -> np.ndarray:
    return run_spmd(inputs, reps=1)

